# revision 1
# baseline (speedup 1.0000x reference)
"""Trainium2 Bass kernel: batched controlled-system dynamics (N = 2^20 states).

Strategy:
  - Pure data parallel over 8 NeuronCores: contiguous slices of the batch axis.
  - Linear part (dv1, F_net/K2, d_xc as functions of x1,v1,x2,v2,xc,sin-term)
    runs on the TensorEngine as a 16-slice block-structured matmul in a
    "feature" SBUF layout [96 = 16 slices x 6 features, 8192]; results are
    copied PSUM->SBUF by the Scalar engine and DMAd out / relaid to ribbon.
  - The friction MLP (1 -> 64 -> 2, tanh + softplus heads) depends only on the
    scalar v2, so it is collapsed host-side into 1D functions of v2:
      kinetic(v2)  = softplus(g0(v2) + b2[0])   (g0 odd for b1 == 0)
      stiction(v2) = softplus(g1(v2) + b2[1])   (only needed for |v2| < 0.01)
    g0 ~ v2 * O(w), O a polynomial in w = affine(v2^2) fit at runtime from the
    actual W1/W2/b1/b2, evaluated as a fused scalar_tensor_tensor Horner chain
    on the Vector engine; softplus = Ln(1 + Exp(.)) on the Scalar engine;
    stiction is linearized around 0 (exact to ~1e-4 on its +-0.01 window).
  - Elementwise work runs in a ribbon layout [128, 1024]: ribbon partition
    16*m + s holds elements [s*8192 + m*1024, +1024) so PSUM h-rows relay to
    contiguous partition blocks.
"""

import numpy as np

# physical system constants (match the reference)
M1, M2 = 1.0, 1.5
K1, K2 = 2.0, 3.0
C1, C2 = 0.5, 0.8
KARNOPP_DV = 0.01
REF_AMP, REF_OMEGA = 0.5, 0.5

N_CORES = 8
N_TOTAL = 1 << 20
N_CORE = N_TOTAL // N_CORES    # 131072
P = 128
F = N_CORE // P                # 1024
MB = P // 16                   # 8 ribbon column-blocks per slice

NSLICE = 16
SLICE_LEN = N_CORE // NSLICE   # 8192
NFEAT = 6                      # x1 v1 x2 v2 xc s
FP = NSLICE * NFEAT            # 96 feature partitions
NOUT = 2                       # dv1, dxc
QCOLS = 2048                   # columns per PSUM ping-pong tile (4 banks)
NQ = SLICE_LEN // QCOLS        # 4 quarters

FIT_TOL = 2.2e-2

_compile_cache = {}


def _softplus(x):
    return np.log1p(np.exp(-np.abs(x))) + np.maximum(x, 0.0)


def _fit_friction(W1, b1, W2, b2, vmax):
    """Fit the 1D collapse of the friction MLP (see module docstring)."""
    W1 = W1.astype(np.float64).reshape(-1)      # [H]
    b1 = b1.astype(np.float64).reshape(-1)
    W2 = W2.astype(np.float64)                  # [H, 2]
    b2 = b2.astype(np.float64).reshape(-1)

    def gg(v, col):
        return np.tanh(np.outer(v, W1) + b1) @ W2[:, col]

    umax = vmax * vmax
    su = 2.0 / umax
    bu = -1.0

    M = 4000
    wn = np.cos(np.pi * (np.arange(M) + 0.5) / M)
    u = (wn + 1.0) / 2.0 * umax
    v = np.sqrt(np.maximum(u, 1e-12))
    gp = gg(v, 0)
    gm = gg(-v, 0)
    E = (gp + gm) / 2.0          # even part of g0 (== 0 when b1 == 0)
    O = (gp - gm) / 2.0 / v      # odd part / v, a function of u (hence w)

    exp_bias = b2[0] + float(np.mean(E))

    vchk = np.linspace(KARNOPP_DV * 0.9, vmax, 30000)
    uchk = vchk * vchk
    wchk = uchk * su + bu
    g0chk = (gg(vchk, 0) - gg(-vchk, 0)) / 2.0

    weight = v + 0.02
    import numpy.polynomial.chebyshev as C
    import numpy.polynomial.polynomial as Pp

    best = None
    for deg in range(6, 17):
        cc = C.chebfit(wn, O, deg, w=weight)
        mono = C.cheb2poly(cc)
        err = np.abs(vchk * Pp.polyval(wchk, mono) - g0chk).max()
        best = (mono, err)
        if err < FIT_TOL:
            break
    mono, err = best

    # stiction limit, linearized at 0:  L(v) = softplus(g1(v) + b2[1])
    d = 1e-4
    g1p = (gg(np.array([d]), 1)[0] - gg(np.array([-d]), 1)[0]) / (2 * d)
    g10 = gg(np.array([0.0]), 1)[0] + b2[1]
    L0 = _softplus(g10)
    sig = 1.0 / (1.0 + np.exp(-g10))
    L1 = sig * g1p

    return dict(oc=mono, su=su, bu=bu, exp_bias=exp_bias, L0=L0, L1=L1,
                fit_err=err, deg=len(mono) - 1)


def _make_wmat(consts):
    """Block lhsT [96, 48]: column m computes output-kind (m//16) for slice
    (m%16) from that slice's 6 feature partitions.

    Features (per slice): x1 v1 x2 v2 xc S, where S = -sin(0.5 t).
    Outputs: 0: dv1, 1: h = F_net/K2, 2: d_xc.
    """
    K = consts["K"]; A = consts["A"]; p = consts["p"]
    B = np.zeros((NFEAT, NOUT), dtype=np.float64)
    # dv1 = (u - K1 x1 - C1 v1 - F_net)/M1, u = A xc + K e, e = -0.5 S - x2
    B[:, 0] = [-(K1 + K2) / M1, -(C1 + C2) / M1, (K2 - K) / M1,
               C2 / M1, A / M1, -0.5 * K / M1]
    # d_xc = e - p*xc = -0.5 S - x2 - p xc
    B[:, 1] = [0.0, 0.0, -1.0, 0.0, -p, -0.5]
    W = np.zeros((FP, NOUT * NSLICE), dtype=np.float32)
    for s in range(NSLICE):
        for o in range(NOUT):
            for f in range(NFEAT):
                W[NSLICE * f + s, o * NSLICE + s] = B[f, o]
    return W


def _build_program(consts):
    """Build the SPMD Bass program (same on all 8 cores)."""
    import concourse.bacc as bacc
    import concourse.mybir as mybir
    import bass_rust as _bass_rust
    from concourse import tile
    from concourse.tile_rust import add_dep_helper
    from concourse.hw_specs import get_activation_tables

    fp32 = mybir.dt.float32
    Alu = mybir.AluOpType
    Act = mybir.ActivationFunctionType

    class _Bacc(bacc.Bacc):
        # The stock table-load pass picks the FIRST act-table set containing
        # each function: Exp -> exp_and_others, Ln -> natural_log (2 loads).
        # Strip exp/ln from those sets so both resolve to the combined
        # natural_log_exp_and_others set (positions/ids preserved).
        def insert_act_table_loads(self):
            has_activation = any(
                isinstance(i, mybir.InstActivation)
                for b in self.main_func.blocks
                for i in b.instructions
            )
            if not has_activation:
                return
            tables = list(get_activation_tables(self.m.arch).items())
            fixed = []
            for name, funcs in tables:
                if name != "trig_and_small":
                    funcs = funcs - {Act.Square, Act.Sign, Act.Abs, Act.Identity, Act.Sin}
                if name != "natural_log_exp_and_others":
                    funcs = funcs - {Act.Exp, Act.Ln}
                fixed.append((name, funcs))
            _bass_rust.insert_act_table_loads(self, fixed)

    c = {k: float(np.float32(v)) for k, v in consts.items() if np.isscalar(v)}
    oc = [float(np.float32(x)) for x in consts["oc"]]
    deg = len(oc) - 1

    nc = _Bacc()

    def reg_const(val):
        v = float(val)
        if (fp32, v) not in nc.const_aps.aps:
            tsr = nc.alloc_sbuf_tensor(f"constu-f32-{len(nc.const_aps.aps)}", [128, 1], fp32)
            nc.gpsimd.memset(tsr.ap(), v)
            nc.const_aps.aps[(fp32, v)] = tsr.ap()

    neg_pi = float(np.float32(-np.pi))
    for v in (neg_pi, c["exp_bias"], c["L0_d"], -c["L0_d"]):
        reg_const(v)
    nc.all_engine_barrier()

    bf16 = mybir.dt.bfloat16
    t_d = nc.dram_tensor("t", [N_CORE], fp32, kind="ExternalInput")
    z_d = nc.dram_tensor("z", [5, N_CORE], fp32, kind="ExternalInput")
    zb_d = nc.dram_tensor("zb", [5, N_CORE], bf16, kind="ExternalInput")
    wm_d = nc.dram_tensor("wmat", [FP, NOUT * NSLICE], bf16, kind="ExternalInput")
    out_d = nc.dram_tensor("out", [5, N_CORE], fp32, kind="ExternalOutput")

    # ribbon view of a flat [N_CORE] vector: partition 16m+s <- elements
    # [s*8192 + m*1024, +1024)
    def rib(ap_row):
        return ap_row.rearrange("(p i) -> p i", p=P)

    t_r = rib(t_d[:])
    z_rib3 = rib(z_d[3, :])
    o_rib3 = rib(out_d[3, :])
    zb_sl = [zb_d[i, :].rearrange("(s q) -> s q", s=NSLICE) for i in range(5)]
    o_sl = [out_d[i, :].rearrange("(s q) -> s q", s=NSLICE) for i in range(5)]

    with tile.TileContext(nc) as tc:
        with tc.tile_pool(name="sb", bufs=1) as pool, \
             tc.tile_pool(name="ps", bufs=1, space="PSUM") as psp:
            def tl(tag, dt=fp32, shape=(P, F)):
                return pool.tile(list(shape), dt, tag=tag, name=tag)

            FEAT = tl("FEAT", bf16, shape=(FP, SLICE_LEN))
            WM = tl("WM", bf16, shape=(FP, NOUT * NSLICE))
            T = tl("T"); V2R = tl("V2R")

            XB4 = tl("XB4", bf16, shape=(P, 4 * F))  # x1|v1|x2|v2 bf16 ribbons
            X1B = XB4[:, 0 * F:1 * F]; V1B = XB4[:, 1 * F:2 * F]
            X2B = XB4[:, 2 * F:3 * F]; V2B = XB4[:, 3 * F:4 * F]
            # ---- loads: sync HWDGE ring (few, large DMAs) ----
            nc.sync.dma_start(out=T[:], in_=t_r)
            nc.sync.dma_start(out=V2R[:], in_=z_rib3)
            nc.sync.dma_start(
                out=XB4[:].rearrange("p (r i) -> p r i", r=4),
                in_=zb_d[0:4, :].rearrange("r (p i) -> p r i", p=P))
            nc.sync.dma_start(
                out=FEAT[0:5 * NSLICE, :],
                in_=zb_d[0:5, :].rearrange("r (s q) -> (r s) q", s=NSLICE))
            nc.sync.dma_start(out=WM[:], in_=wm_d[:])

            # ---- ACT phase 1 (table set: trig_and_small) ----
            U = tl("U")
            nc.scalar.activation(U[:], V2R[:], Act.Square)
            S = tl("S", bf16)  # S = sin(0.5 t - pi) = -sin(0.5 t)
            i_sin = nc.scalar.activation(S[:], T[:], Act.Sin, bias=neg_pi, scale=0.5)

            # move S into the feature layout: FEAT[80+s, m*1024+i] = S[8s+m, i]
            for m in range(MB):
                nc.gpsimd.dma_start(out=FEAT[5 * NSLICE:6 * NSLICE, m * F:(m + 1) * F],
                                  in_=S[m::MB, :])

            SGN = tl("SGN")
            i_sgn = nc.scalar.activation(SGN[:], V2R[:], Act.Sign)
            AV = tl("AV")
            i_av = nc.scalar.activation(AV[:], V2R[:], Act.Abs)
            for bi in (i_sgn, i_av):
                add_dep_helper(bi.ins, i_sin.ins, sync=False, reason="act table order")
            LP = tl("LP"); NLP = tl("NLP")
            i_lp = nc.scalar.activation(LP[:], V2R[:], Act.Identity,
                                        bias=c["L0_d"], scale=c["L1_d"])
            i_nlp = nc.scalar.activation(NLP[:], V2R[:], Act.Identity,
                                         bias=-c["L0_d"], scale=-c["L1_d"])
            for bi in (i_lp, i_nlp):
                add_dep_helper(bi.ins, i_sin.ins, sync=False, reason="act table order")

            # ---- polynomial for g0 (odd part of the kinetic head) ----
            W = tl("W")
            nc.vector.tensor_scalar(W[:], U[:], c["su"], c["bu"], Alu.mult, Alu.add)
            acc = tl("ACCa")
            nc.vector.tensor_single_scalar(acc[:], W[:], oc[deg], Alu.mult)
            flip = False
            for k in range(deg - 1, 0, -1):
                nxt = tl("ACCb" if not flip else "ACCa")
                nc.vector.scalar_tensor_tensor(nxt[:], acc[:], oc[k], W[:], Alu.add, Alu.mult)
                acc = nxt
                flip = not flip
            G0 = tl("G0")
            nc.vector.scalar_tensor_tensor(G0[:], acc[:], oc[0], V2R[:], Alu.add, Alu.mult)

            # ---- ACT phase 2 (table set: natural_log_exp_and_others) ----
            Q = tl("Q")
            nc.scalar.activation(Q[:], G0[:], Act.Exp, bias=c["exp_bias"])
            KIN = tl("KIN")  # softplus(g0 + b2[0]) = ln(1 + exp(...))
            nc.scalar.activation(KIN[:], Q[:], Act.Ln, bias=1.0)

            # ---- h = F_net/K2 on DVE from bf16 ribbons ----
            FD1 = tl("FD1")
            nc.vector.tensor_tensor(FD1[:], X1B, X2B, Alu.subtract)
            FD2 = tl("FD2")
            nc.vector.tensor_tensor(FD2[:], V1B, V2B, Alu.subtract)
            H = tl("H")
            nc.vector.scalar_tensor_tensor(H[:], FD2[:], C2 / K2, FD1[:], Alu.mult, Alu.add)

            # ---- friction clip bounds ----
            MASK = pool.tile([P, F], mybir.dt.uint8, tag="MASK", name="MASK")
            nc.vector.tensor_single_scalar(MASK[:], AV[:], KARNOPP_DV, Alu.is_lt)
            MX = tl("MX")
            nc.vector.tensor_tensor(MX[:], H[:], NLP[:], Alu.max)
            MM = tl("MM")
            nc.vector.tensor_tensor(MM[:], MX[:], LP[:], Alu.min)
            HS = tl("HS")    # F_net / M2
            nc.vector.tensor_single_scalar(HS[:], H[:], K2 / M2, Alu.mult)

            # ---- friction select + dv2 ----
            PHI = tl("PHI")  # kinetic/K2 * sign(v2) = -F_kinetic/K2
            nc.vector.scalar_tensor_tensor(PHI[:], KIN[:], 1.0 / K2, SGN[:], Alu.mult, Alu.mult)
            nc.vector.copy_predicated(PHI[:], MASK[:], MM[:])
            DV2 = tl("DV2")  # (F_net + F_friction)/M2 = HS - (K2/M2)*PHI
            nc.vector.scalar_tensor_tensor(DV2[:], PHI[:], -K2 / M2, HS[:], Alu.mult, Alu.add)
            nc.sync.dma_start(out=o_rib3, in_=DV2[:])

            # ---- TensorEngine: dv1 / d_xc, pipelined by quarter ----
            PS = [psp.tile([NOUT * NSLICE, QCOLS], fp32, name=f"PS{i}", tag=f"PS{i}")
                  for i in range(2)]
            STG = tl("STG", shape=(NOUT * NSLICE, SLICE_LEN))
            for q in range(NQ):
                ps = PS[q % 2]
                base = q * QCOLS
                for cchunk in range(QCOLS // 512):
                    col = base + cchunk * 512
                    nc.tensor.matmul(ps[:, cchunk * 512:(cchunk + 1) * 512],
                                     WM[:], FEAT[:, col:col + 512],
                                     start=True, stop=True)
                # PSUM -> SBUF staging (ScalarE sits next to PSUM)
                nc.scalar.activation(STG[:, base:base + QCOLS], ps[:], Act.Copy)
                if q % 2 == 1:
                    # store a 4096-column half of each output row-group
                    hb = (q // 2) * 2 * QCOLS
                    nc.gpsimd.dma_start(
                        out=o_sl[1][:, hb:hb + 2 * QCOLS],
                        in_=STG[0:NSLICE, hb:hb + 2 * QCOLS])
                    nc.gpsimd.dma_start(
                        out=o_sl[4][:, hb:hb + 2 * QCOLS],
                        in_=STG[NSLICE:2 * NSLICE, hb:hb + 2 * QCOLS])

            # passthrough rows: dx1 = v1 (DRAM->DRAM), dx2 = v2 (fp32 ribbon)
            nc.gpsimd.dma_start(out=out_d[0, :], in_=z_d[1, :])
            nc.sync.dma_start(out=rib(out_d[2, :]), in_=V2R[:])

    nc.finalize()
    return nc


def _prepare(inputs):
    """Host-side constant folding + program build (cached on weight values)."""
    logK = np.float32(inputs["logK"]); logz = np.float32(inputs["logz"])
    logp = np.float32(inputs["logp"])
    W1 = np.asarray(inputs["W1"], dtype=np.float32)
    b1 = np.asarray(inputs["b1"], dtype=np.float32)
    W2 = np.asarray(inputs["W2"], dtype=np.float32)
    b2 = np.asarray(inputs["b2"], dtype=np.float32)
    v2 = np.asarray(inputs["z"][3], dtype=np.float32)
    vmax = float(np.abs(v2).max()) * 1.02 + 1e-3

    key = (logK.tobytes(), logz.tobytes(), logp.tobytes(), W1.tobytes(),
           b1.tobytes(), W2.tobytes(), b2.tobytes(), round(vmax, 3))
    if key in _compile_cache:
        return _compile_cache[key]

    K = np.float32(np.exp(logK))
    z_ctrl = np.float32(np.exp(logz))
    p_ctrl = np.float32(np.exp(logp))
    A = np.float32(K * (z_ctrl - p_ctrl))

    fit = _fit_friction(W1, b1, W2, b2, vmax)

    consts = dict(
        K=float(K), p=float(p_ctrl), A=float(A),
        su=fit["su"], bu=fit["bu"], exp_bias=fit["exp_bias"],
        L0_d=fit["L0"] / K2, L1_d=fit["L1"] / K2,
        oc=fit["oc"],
    )
    wmat = _make_wmat(consts)
    nc = _build_program(consts)
    _compile_cache[key] = (nc, fit, wmat)
    return nc, fit, wmat


def _run(inputs, trace=False):
    from concourse.bass_utils import run_bass_kernel_spmd

    nc, _fit, wmat = _prepare(inputs)

    import ml_dtypes
    t = np.ascontiguousarray(np.asarray(inputs["t"], dtype=np.float32))
    z = np.ascontiguousarray(np.asarray(inputs["z"], dtype=np.float32))
    zb = z.astype(ml_dtypes.bfloat16)
    wmat_b = wmat.astype(ml_dtypes.bfloat16)
    in_maps = []
    for i in range(N_CORES):
        sl = slice(i * N_CORE, (i + 1) * N_CORE)
        in_maps.append({"t": np.ascontiguousarray(t[sl]),
                        "z": np.ascontiguousarray(z[:, sl]),
                        "zb": np.ascontiguousarray(zb[:, sl]),
                        "wmat": wmat_b})

    res = run_bass_kernel_spmd(nc, in_maps, core_ids=list(range(N_CORES)),
                               trace=trace)
    out = np.empty((5, N_TOTAL), dtype=np.float32)
    for i in range(N_CORES):
        out[:, i * N_CORE:(i + 1) * N_CORE] = res.results[i]["out"]
    return out, res


def kernel(**inputs):
    out, _res = _run(inputs, trace=False)
    return out

